# revision 5
# baseline (speedup 1.0000x reference)
"""4-bit comparator (a>b, a==b) over [8388608, 4] binary spike inputs.

Strategy: rows are data-parallel across 8 NeuronCores. Host losslessly
repacks each operand's 4 bits into one byte (a = 8a3+4a2+2a1+a0 in
[0,15]; b is sent biased as b' = 16-b in [1,16]); pairs of adjacent
rows travel as one int16. On-core the DVE adds the int16 streams in 2x
mode -- per-byte lane sums a+b' = (a-b)+16 stay in [1,31], so no carry
ever crosses the byte boundary. The comparison result s in {-1,0,1}
(s=1 iff a>b, s=0 iff a==b) is emitted as int8: the ACT engine
evaluates Sign(lane-16) for chunks 0-2 while the DVE clamps chunk 3
via (x-16) max -1 min 1, so the last chunk never waits on the ACT
stream. Host decodes gt = (s==1), eq = (s==0).

All tiles are fully buffered (4 MiB SBUF) so no buffer-reuse waits
exist; av DMAs issue from the sync queue and bv DMAs from the gpsimd
queue in parallel. DMA triggers that read engine-written tiles are
gated on the producing instruction's semaphore (the sequencer executes
dma_start triggers without waiting for queued engine instructions).

HBM traffic per core: 2 MiB in + 1 MiB out (vs 320 MiB f32 full I/O
across the chip).
"""

import sys

if "/opt/trn_rl_repo" not in sys.path:
    sys.path.insert(0, "/opt/trn_rl_repo")

import numpy as np

N_ROWS = 8_388_608
N_CORES = 8
R = N_ROWS // N_CORES          # rows per core = 1,048,576
P = 128                        # SBUF partitions
MPP = R // P                   # rows (bytes) per partition = 8192
W16 = MPP // 2                 # int16 words per partition = 4096
NCH = 4                        # pipeline chunks per core
CH16 = W16 // NCH              # int16 per partition per chunk (1024 = 2KiB)
CH8 = MPP // NCH               # bytes per partition per chunk (2048)

_CACHE = {}


def _build():
    import concourse.bass as bass
    import concourse.mybir as mybir

    nc = bass.Bass(trn_type="TRN2")
    i16 = mybir.dt.int16
    i8 = mybir.dt.int8
    u8 = mybir.dt.uint8
    f32 = mybir.dt.float32
    AluOp = mybir.AluOpType
    AF = mybir.ActivationFunctionType

    av = nc.dram_tensor("av", [P, W16], i16, kind="ExternalInput")
    bv = nc.dram_tensor("bv", [P, W16], i16, kind="ExternalInput")
    out = nc.dram_tensor("out", [P, MPP], i8, kind="ExternalOutput")

    from contextlib import ExitStack
    with ExitStack() as ctx:
        ec = ctx.enter_context
        av_t = ec(nc.sbuf_tensor("av_t", [P, W16], i16))
        bv_t = ec(nc.sbuf_tensor("bv_t", [P, W16], i16))
        tt = ec(nc.sbuf_tensor("tt", [P, W16], i16))
        st = ec(nc.sbuf_tensor("st", [P, MPP], i8))
        bias_t = ec(nc.sbuf_tensor("bias_t", [P, 1], f32))
        dummy_i = ec(nc.sbuf_tensor("dummy_i", [P, 16], u8))
        dummy_o = ec(nc.sbuf_tensor("dummy_o", [P, 16], i8))
        s_ina = ec(nc.semaphore(name="s_ina"))
        s_inb = ec(nc.semaphore(name="s_inb"))
        s_add = ec(nc.semaphore(name="s_add"))
        s_cmp = ec(nc.semaphore(name="s_cmp"))
        s_ve = ec(nc.semaphore(name="s_ve"))
        s_out = ec(nc.semaphore(name="s_out"))
        s_pre = ec(nc.semaphore(name="s_pre"))
        block = ec(nc.Block())

        @block.sync
        def _(sync):
            for c in range(NCH):
                sl = slice(c * CH16, (c + 1) * CH16)
                sync.dma_start(av_t[:, sl], av[:, sl]).then_inc(s_ina, 16)
            # chunk 3's out (DVE-clamped); gate on the DVE's TS completion
            sync.wait_ge(s_ve, 1)
            osl = slice(3 * CH8, 4 * CH8)
            sync.dma_start(out[:, osl], st[:, osl]).then_inc(s_out, 16)
            sync.wait_ge(s_out, 16 * NCH)

        @block.gpsimd
        def _(gps):
            for c in range(NCH):
                sl = slice(c * CH16, (c + 1) * CH16)
                gps.dma_start(bv_t[:, sl], bv[:, sl]).then_inc(s_inb, 16)

        @block.vector
        def _(dve):
            # bias constant for ACT + dummy input for the act-table
            # prefetch activation
            nc.vector.memset(bias_t[:], -16.0).then_inc(s_pre, 1)
            nc.vector.memset(dummy_i[:], 0).then_inc(s_pre, 1)
            for c in range(NCH):
                dve.wait_ge(s_ina, 16 * (c + 1))
                dve.wait_ge(s_inb, 16 * (c + 1))
                sl = slice(c * CH16, (c + 1) * CH16)
                nc.vector.tensor_tensor(
                    tt[:, sl], av_t[:, sl], bv_t[:, sl], AluOp.add
                ).then_inc(s_add, 1)
            # chunk 3's compare on the DVE: s = (x-16) clamped to [-1,1]
            bsl = slice(3 * CH8, 4 * CH8)
            tb = tt[:].bitcast(u8)
            nc.vector.tensor_scalar(
                out=st[:, bsl], in0=tb[:, bsl], scalar1=16, scalar2=-1,
                op0=AluOp.subtract, op1=AluOp.max,
            )
            nc.vector.tensor_scalar(
                out=st[:, bsl], in0=st[:, bsl], scalar1=1, scalar2=None,
                op0=AluOp.min,
            ).then_inc(s_ve, 1)

        @block.scalar
        def _(act):
            # issue one tiny Sign first so the activation-table load
            # (~2.7us) overlaps the chunk-0 DMA+add instead of
            # serializing after it
            act.wait_ge(s_pre, 2)
            nc.scalar.activation(dummy_o[:], dummy_i[:], AF.Sign, bias=bias_t[:])
            tb = tt[:].bitcast(u8)
            for c in range(NCH - 1):
                act.wait_ge(s_add, c + 1)
                bsl = slice(c * CH8, (c + 1) * CH8)
                nc.scalar.activation(
                    st[:, bsl], tb[:, bsl], AF.Sign, bias=bias_t[:],
                ).then_inc(s_cmp, 1)
                # gate the out-DMA on the ACTIVATE's completion (sequencer
                # runs dma_start triggers ahead of queued engine instrs)
                act.wait_ge(s_cmp, c + 1)
                act.dma_start(out[:, bsl], st[:, bsl]).then_inc(s_out, 16)

    return nc


def _get_nc():
    if "nc" not in _CACHE:
        _CACHE["nc"] = _build()
    return _CACHE["nc"]


def kernel(A, B, trace=False):
    from concourse import bass_utils

    A = np.asarray(A)
    B = np.asarray(B)
    assert A.shape == (N_ROWS, 4) and B.shape == (N_ROWS, 4), (A.shape, B.shape)

    w = np.array([8.0, 4.0, 2.0, 1.0], dtype=np.float32)
    va = (A @ w).astype(np.uint8)            # value of a, 0..15
    vb = (16.0 - (B @ w)).astype(np.uint8)   # 16 - value of b, 1..16

    in_maps = []
    for i in range(N_CORES):
        sl = slice(i * R, (i + 1) * R)
        in_maps.append({
            "av": va[sl].reshape(P, MPP).view(np.int16),
            "bv": vb[sl].reshape(P, MPP).view(np.int16),
        })

    nc = _get_nc()
    res = bass_utils.run_bass_kernel_spmd(
        nc, in_maps, core_ids=list(range(N_CORES)), trace=trace,
    )
    _CACHE["last_results"] = res

    gt = np.empty((N_ROWS,), dtype=np.float32)
    eq = np.empty((N_ROWS,), dtype=np.float32)
    for i in range(N_CORES):
        s = np.asarray(res.results[i]["out"]).reshape(R)  # int8 {-1,0,1}
        sl = slice(i * R, (i + 1) * R)
        gt[sl] = (s == 1)
        eq[sl] = (s == 0)
    return gt.reshape(N_ROWS, 1), eq.reshape(N_ROWS, 1)
